# revision 5
# baseline (speedup 1.0000x reference)
"""Trainium2 Bass kernel for MinimalLightningIndexer (v2).

out[b,t,s] = relu((x@Wq)[b,t] . (x@Wk)[b,s]) * (x@Ww)[b,t]

Sharding: 8 cores = 4 batches x 2 token-halves. Each core receives ONLY
its own half of x[b] (transposed, 8.4 MB bf16), projects [k|q|w] for its
2048 tokens, and scores its 2048 queries against all 4096 keys. The
16-dim key rows are exchanged with the sibling core via two pipelined
pairwise AllGathers (2 x 32 KB). Instead of an indirect select, the
gathered [32, 1024] key block is masked on GPSIMD by a per-core 0/1 row
mask (qmask input) and scored against a q-stationary duplicated into
both 16-row slots — SPMD-uniform, no data-dependent control flow.

v2 schedule changes vs v1 (trace-driven):
 - PE program interleaves score phases with projections (v1 serialized
   warmup -> all projections -> all scores; zero output bytes left the
   core for the first 36 us of a 116 us run).
 - Warmup shortened: v1's 56-matmul chain held the PE head for 14 us,
   delaying projections far past their input-ready time.
 - Collectives fire as soon as their key halves exist; small extracts
   ride the SWDGE queue so the two HWDGE rings carry only bulk bytes
   (v1's AllGather doorbell was queue-blocked until 49 us; sibling
   keys landed at 62/75 us and stretched the B-phase to 109 us).
 - Output written as 64 x 256 KB DMAs split across both HWDGE rings,
   queued while the input is still streaming.
 - Postproc (relu * gate) split DVE/ACT: every 3rd group fused on DVE
   (tensor_scalar MAX+MULT from PSUM), the rest ACT relu + DVE bf16
   in-place multiply at 4x mode.

PE clock note: score matmuls contract only 16 idx dims; the stationary
q operand is zero-padded to K=128 so the HAM activity monitor sees a
busy array (1.2 -> 2.4 GHz), and warm-up matmuls bridge the input load.
Every rhs row under a nonzero stationary row is memset (0 * NaN -> NaN
would poison PSUM).
"""

import os
import sys

if "/opt/trn_rl_repo" not in sys.path:
    sys.path.insert(0, "/opt/trn_rl_repo")

import numpy as np

import concourse.bacc as bacc
import concourse.bass as bass
import concourse.mybir as mybir
import concourse.tile as tile
from concourse.bass_utils import run_bass_kernel_spmd

B, S, D = 4, 4096, 2048
IDX = 16
N_CORES = 8
T = S // 2           # own tokens per core
DC = D // 128        # 16 d-chunks
NG = 4               # projection groups (512 tokens each)
W33 = 2 * IDX + 1    # [k | q | w] projection width

WARM_N = int(os.environ.get("K_WARM_N", "48"))     # PE warm-up matmuls
FUSE_MOD = int(os.environ.get("K_FUSE_MOD", "3"))  # gi % FUSE_MOD == 0 -> fused

_CACHE = {}


def _build_nc():
    if "nc" in _CACHE:
        return _CACHE["nc"]
    f32 = mybir.dt.float32
    bf16 = mybir.dt.bfloat16
    nc = bacc.Bacc("TRN2", target_bir_lowering=False, debug=False,
                   num_devices=N_CORES)

    xh = nc.dram_tensor("xh", [NG * 128, DC * 512], bf16,
                        kind="ExternalInput").ap()
    wkqw = nc.dram_tensor("wkqw", [128, DC * W33], bf16,
                          kind="ExternalInput").ap()
    qmask = nc.dram_tensor("qmask", [2 * IDX, 1], f32,
                           kind="ExternalInput").ap()
    o = nc.dram_tensor("o", [T, S], bf16, kind="ExternalOutput").ap()

    groups = [[2 * i, 2 * i + 1] for i in range(N_CORES // 2)]

    with tile.TileContext(nc) as tc:
        with (
            tc.tile_pool(name="const", bufs=1) as cpool,
            tc.tile_pool(name="slab", bufs=4) as slab_pool,
            tc.tile_pool(name="osb", bufs=20) as out_pool,
            tc.tile_pool(name="pj", bufs=1, space="PSUM") as pj_pool,
            tc.tile_pool(name="ps", bufs=3, space="PSUM") as ps_pool,
            tc.tile_pool(name="pw", bufs=1, space="PSUM") as pw_pool,
            tc.tile_pool(name="dram", bufs=1, space="DRAM") as dpool,
        ):
            # --- persistent tiles ---
            wkqw_sb = cpool.tile([128, DC * W33], bf16, tag="wkqw_sb")
            qmask_sb = cpool.tile([2 * IDX, 1], f32, tag="qmask_sb")
            s33_sb = cpool.tile([128, T], bf16, tag="s33_sb")
            qT_sb = cpool.tile([128, T], bf16, tag="qT_sb")
            qT2_sb = cpool.tile([128, T], bf16, tag="qT2_sb")
            kgs_sb = [cpool.tile([128, 1024], bf16, name=f"kgs{st}",
                                 tag=f"kgs{st}") for st in range(2)]
            w_colb = cpool.tile([128, T // 128], bf16, tag="w_colb")
            w_col = cpool.tile([128, T // 128], f32, tag="w_col")
            warm_sb = cpool.tile([128, 512], bf16, tag="warm_sb")

            kin = [dpool.tile([IDX, 1024], bf16, name=f"kin{st}",
                              tag=f"kin{st}") for st in range(2)]
            kg = [dpool.tile([2 * IDX, 1024], bf16, name=f"kg{st}",
                             tag=f"kg{st}") for st in range(2)]

            # engine ops at a nonzero partition offset may span at most
            # 32 partitions — memset those regions in 32-row chunks
            nc.vector.memset(warm_sb[:], 0)
            nc.vector.memset(qT_sb[:], 0)
            nc.vector.memset(qT2_sb[:], 0)
            for p0 in range(32, 128, 32):
                nc.vector.memset(s33_sb[p0:p0 + 32, :], 0)
                for st in range(2):
                    nc.vector.memset(kgs_sb[st][p0:p0 + 32, :], 0)

            # PE warm-up bridges the first input slab's load time
            if WARM_N:
                pwarm = pw_pool.tile([128, 512], f32, tag="pwarm")
                for wi in range(WARM_N):
                    nc.tensor.matmul(
                        pwarm[:], warm_sb[:, 0:128], warm_sb[:],
                        start=(wi == 0), stop=(wi == WARM_N - 1),
                    )

            # small persistent loads on the ACT ring (ahead of slab
            # halves so they land first)
            nc.scalar.dma_start(out=wkqw_sb[:], in_=wkqw)
            nc.scalar.dma_start(out=qmask_sb[:], in_=qmask)

            # input slabs: halves alternate across the two HWDGE rings;
            # ONLY bulk bytes ride these rings (small extracts would be
            # FIFO-blocked behind megabytes of queued input)
            slabs = []
            half = DC * 256
            for g in range(NG):
                slab = slab_pool.tile([128, DC * 512], bf16, tag="slab")
                nc.sync.dma_start(
                    out=slab[:, 0:half],
                    in_=xh[g * 128:(g + 1) * 128, 0:half])
                nc.scalar.dma_start(
                    out=slab[:, half:2 * half],
                    in_=xh[g * 128:(g + 1) * 128, half:2 * half])
                slabs.append(slab)

            # --- emission helpers ---
            st_gi = [0]

            def proj_mm(g):
                slab_v = slabs[g][:].rearrange("p (kd t) -> p kd t", kd=DC)
                pj = pj_pool.tile([W33, 512], f32, tag="pj")
                for kd in range(DC):
                    nc.tensor.matmul(
                        pj[:],
                        wkqw_sb[:, kd * W33:(kd + 1) * W33],
                        slab_v[:, kd, :],
                        start=(kd == 0), stop=(kd == DC - 1),
                    )
                c0, c1 = g * 512, (g + 1) * 512
                nc.vector.tensor_copy(s33_sb[0:W33, c0:c1], pj[:])

            def qT_extract(g):
                # engine reads need 32-aligned partition offsets; DMAs
                # don't — pull q rows out of s33 by SWDGE
                c0, c1 = g * 512, (g + 1) * 512
                nc.gpsimd.dma_start(
                    out=qT_sb[0:IDX, c0:c1], in_=s33_sb[IDX:2 * IDX, c0:c1])

            def kin_extract(g):
                c0, c1 = g * 512, (g + 1) * 512
                st, hf = g // 2, g % 2
                nc.gpsimd.dma_start(
                    out=kin[st][:, hf * 512:(hf + 1) * 512],
                    in_=s33_sb[0:IDX, c0:c1])

            def w_extract(g):
                c0 = g * 512
                for gi_ in range(4):
                    t0 = c0 + gi_ * 128
                    nc.gpsimd.dma_start(
                        out=w_colb[:, g * 4 + gi_:g * 4 + gi_ + 1],
                        in_=s33_sb[2 * IDX:W33, t0:t0 + 128],
                    )

            def qT2_extract(g):
                c0, c1 = g * 512, (g + 1) * 512
                nc.gpsimd.dma_start(
                    out=qT2_sb[0:IDX, c0:c1], in_=s33_sb[IDX:2 * IDX, c0:c1])
                nc.gpsimd.dma_start(
                    out=qT2_sb[IDX:2 * IDX, c0:c1],
                    in_=s33_sb[IDX:2 * IDX, c0:c1])

            def w_cast(g):
                nc.vector.tensor_copy(
                    w_col[:, g * 4:(g + 1) * 4],
                    w_colb[:, g * 4:(g + 1) * 4])

            def collective(st):
                nc.gpsimd.collective_compute(
                    "AllGather",
                    mybir.AluOpType.bypass,
                    replica_groups=groups,
                    ins=[kin[st].opt()],
                    outs=[kg[st].opt()],
                )

            def kgs_load(st, ring):
                ring.dma_start(out=kgs_sb[st][0:2 * IDX, :], in_=kg[st][:])

            def kgs_mask(st):
                # sibling-slot select: zero own rows via per-core 0/1
                # mask data; program stays uniform
                nc.gpsimd.tensor_scalar_mul(
                    out=kgs_sb[st][0:2 * IDX, :],
                    in0=kgs_sb[st][0:2 * IDX, :],
                    scalar1=qmask_sb[:, 0:1],
                )

            def score_block(i, stat, rhs, rb, ob, ring):
                ps = ps_pool.tile([128, 1024], f32, tag="ps")
                for jj in range(2):
                    j0 = rb + jj * 512
                    nc.tensor.matmul(
                        ps[:, jj * 512:(jj + 1) * 512],
                        stat[:, i * 128:(i + 1) * 128],
                        rhs[:, j0:j0 + 512],
                        start=True, stop=True,
                    )
                osb = out_pool.tile([128, 1024], bf16, tag="osb")
                gi = st_gi[0]
                st_gi[0] += 1
                if gi % FUSE_MOD == 0:
                    nc.vector.tensor_scalar(
                        out=osb[:],
                        in0=ps[:],
                        scalar1=0.0,
                        scalar2=w_col[:, i:i + 1],
                        op0=mybir.AluOpType.max,
                        op1=mybir.AluOpType.mult,
                    )
                else:
                    nc.scalar.activation(
                        osb[:], ps[:], mybir.ActivationFunctionType.Relu)
                    nc.vector.tensor_scalar_mul(
                        out=osb[:], in0=osb[:], scalar1=w_col[:, i:i + 1])
                ring.dma_start(
                    out=o[i * 128:(i + 1) * 128, ob:ob + 1024],
                    in_=osb[:],
                )

            # --- program ---
            # PE:     warm | g0 | g1 | P1a | g2 | P1b | P2a | g3 | P2b
            #         | P3 | P4 | P5
            # GPSIMD: qT0 kin0 w0 qT2(0) | qT1 kin1 CC0 w1 qT2(1)
            #         | qT2 kin2 w2 qT2(2) | qT3 kin3 CC1 w3 qT2(3)
            #         | mask0 mask1
            # sync:   slabs-a | P1a P1b P2a outs | kgs0 | P2b P4 outs
            # scalar: wkqw qmask slabs-b | P3 outs | kgs1 | P5 outs
            proj_mm(0)
            qT_extract(0); kin_extract(0); w_extract(0)
            w_cast(0)
            qT2_extract(0)

            proj_mm(1)
            qT_extract(1); kin_extract(1)
            collective(0)

            for i in range(0, 4):                       # P1a: A0 i0-3
                score_block(i, qT_sb, s33_sb, 0, 0, nc.sync)
            w_extract(1); w_cast(1)
            qT2_extract(1)

            proj_mm(2)
            qT_extract(2); kin_extract(2); w_extract(2)

            for i in range(4, 8):                       # P1b: A0 i4-7
                score_block(i, qT_sb, s33_sb, 0, 0, nc.sync)
            w_cast(2)
            qT2_extract(2)

            for i in range(8, 12):                      # P2a: A0 i8-11
                score_block(i, qT_sb, s33_sb, 0, 0, nc.sync)

            proj_mm(3)
            qT_extract(3); kin_extract(3)
            collective(1)
            w_extract(3); w_cast(3)
            qT2_extract(3)

            kgs_load(0, nc.sync)
            kgs_mask(0)

            for i in range(12, 16):                     # P2b: A0 i12-15
                score_block(i, qT_sb, s33_sb, 0, 0, nc.sync)
            for i in range(16):                         # P3: A1
                score_block(i, qT_sb, s33_sb, 1024, 1024, nc.scalar)

            kgs_load(1, nc.scalar)
            kgs_mask(1)

            for i in range(16):                         # P4: B0
                score_block(i, qT2_sb, kgs_sb[0], 0, 2048, nc.sync)
            for i in range(16):                         # P5: B1
                score_block(i, qT2_sb, kgs_sb[1], 0, 3072, nc.scalar)

    nc.compile()
    _CACHE["nc"] = nc
    return nc


def _make_in_maps(x, Wq, Wk, Ww):
    import ml_dtypes
    bf = ml_dtypes.bfloat16
    w33 = np.concatenate([Wk, Wq, Ww], axis=1).astype(bf)       # [D, 33]
    wkqw = np.ascontiguousarray(
        w33.reshape(DC, 128, W33).transpose(1, 0, 2).reshape(128, DC * W33))
    xbf = x.astype(bf)
    in_maps = []
    for c in range(N_CORES):
        b, h = c // 2, c % 2
        own = xbf[b, h * T:(h + 1) * T, :]                       # [T, D]
        xt = own.T                                               # [D, T]
        xs = np.ascontiguousarray(
            xt.reshape(DC, 128, NG, 512)
            .transpose(2, 1, 0, 3).reshape(NG * 128, DC * 512))
        qm = np.zeros((2 * IDX, 1), dtype=np.float32)
        qm[(1 - h) * IDX:(2 - h) * IDX, 0] = 1.0   # keep sibling's rows
        in_maps.append({"xh": xs, "wkqw": wkqw, "qmask": qm})
    return in_maps


def _assemble(results):
    out = np.empty((B, S, S), dtype=np.float32)
    for c in range(N_CORES):
        b, h = c // 2, c % 2
        oc = np.asarray(results[c]["o"], dtype=np.float32)
        r0 = h * T
        out[b, r0:r0 + T, h * T:(h + 1) * T] = oc[:, 0:T]
        out[b, r0:r0 + T, (1 - h) * T:(2 - h) * T] = oc[:, T:S]
    return out


def kernel(x, Wq, Wk, Ww, _trace_kwargs=None):
    nc = _build_nc()
    in_maps = _make_in_maps(np.asarray(x, dtype=np.float32),
                            np.asarray(Wq, dtype=np.float32),
                            np.asarray(Wk, dtype=np.float32),
                            np.asarray(Ww, dtype=np.float32))
    kw = _trace_kwargs or {}
    res = run_bass_kernel_spmd(nc, in_maps, list(range(N_CORES)), **kw)
    out = _assemble(res.results)
    if _trace_kwargs is not None:
        return out, res
    return out


# revision 9
# speedup vs baseline: 1.9132x; 1.9132x over previous
"""Trainium2 Bass kernel for MinimalLightningIndexer (v2).

out[b,t,s] = relu((x@Wq)[b,t] . (x@Wk)[b,s]) * (x@Ww)[b,t]

Sharding: 8 cores = 4 batches x 2 token-halves. Each core receives ONLY
its own half of x[b] (transposed, 8.4 MB bf16), projects [k|q|w] for its
2048 tokens, and scores its 2048 queries against all 4096 keys. The
16-dim key rows are exchanged with the sibling core via two pipelined
pairwise AllGathers (2 x 32 KB). Instead of an indirect select, the
gathered [32, 1024] key block is masked on GPSIMD by a per-core 0/1 row
mask (qmask input) and scored against a q-stationary duplicated into
both 16-row slots — SPMD-uniform, no data-dependent control flow.

v2 schedule changes vs v1 (trace-driven):
 - PE program interleaves score phases with projections (v1 serialized
   warmup -> all projections -> all scores; zero output bytes left the
   core for the first 36 us of a 116 us run).
 - Warmup shortened: v1's 56-matmul chain held the PE head for 14 us,
   delaying projections far past their input-ready time.
 - Collectives fire as soon as their key halves exist; small extracts
   ride the SWDGE queue so the two HWDGE rings carry only bulk bytes
   (v1's AllGather doorbell was queue-blocked until 49 us; sibling
   keys landed at 62/75 us and stretched the B-phase to 109 us).
 - Output written as 64 x 256 KB DMAs split across both HWDGE rings,
   queued while the input is still streaming.
 - Postproc (relu * gate) split DVE/ACT: every 3rd group fused on DVE
   (tensor_scalar MAX+MULT from PSUM), the rest ACT relu + DVE bf16
   in-place multiply at 4x mode.

PE clock note: score matmuls contract only 16 idx dims; the stationary
q operand is zero-padded to K=128 so the HAM activity monitor sees a
busy array (1.2 -> 2.4 GHz), and warm-up matmuls bridge the input load.
Every rhs row under a nonzero stationary row is memset (0 * NaN -> NaN
would poison PSUM).
"""

import os
import sys

if "/opt/trn_rl_repo" not in sys.path:
    sys.path.insert(0, "/opt/trn_rl_repo")

import numpy as np

import concourse.bacc as bacc
import concourse.bass as bass
import concourse.mybir as mybir
import concourse.tile as tile
from concourse.bass_utils import run_bass_kernel_spmd

B, S, D = 4, 4096, 2048
IDX = 16
N_CORES = 8
T = S // 2           # own tokens per core
DC = D // 128        # 16 d-chunks
NG = 4               # projection groups (512 tokens each)
W33 = 2 * IDX + 1    # [k | q | w] projection width

WARM_N = int(os.environ.get("K_WARM_N", "48"))     # PE warm-up matmuls
FUSE_MOD = int(os.environ.get("K_FUSE_MOD", "3"))  # gi % FUSE_MOD == 0 -> fused

_CACHE = {}


def _build_nc():
    if "nc" in _CACHE:
        return _CACHE["nc"]
    f32 = mybir.dt.float32
    bf16 = mybir.dt.bfloat16
    nc = bacc.Bacc("TRN2", target_bir_lowering=False, debug=False,
                   num_devices=N_CORES)

    xh = nc.dram_tensor("xh", [NG * 128, DC * 512], bf16,
                        kind="ExternalInput").ap()
    wkqw = nc.dram_tensor("wkqw", [128, DC * W33], bf16,
                          kind="ExternalInput").ap()
    qmask = nc.dram_tensor("qmask", [2 * IDX, 1], f32,
                           kind="ExternalInput").ap()
    o = nc.dram_tensor("o", [T, S], bf16, kind="ExternalOutput").ap()

    groups = [[2 * i, 2 * i + 1] for i in range(N_CORES // 2)]

    with tile.TileContext(nc) as tc:
        with (
            tc.tile_pool(name="const", bufs=1) as cpool,
            tc.tile_pool(name="slab", bufs=4) as slab_pool,
            tc.tile_pool(name="osb", bufs=20) as out_pool,
            tc.tile_pool(name="pj", bufs=1, space="PSUM") as pj_pool,
            tc.tile_pool(name="ps", bufs=3, space="PSUM") as ps_pool,
            tc.tile_pool(name="pw", bufs=1, space="PSUM") as pw_pool,
            tc.tile_pool(name="dram", bufs=1, space="DRAM") as dpool,
        ):
            # --- persistent tiles ---
            wkqw_sb = cpool.tile([128, DC * W33], bf16, tag="wkqw_sb")
            qmask_sb = cpool.tile([2 * IDX, 1], f32, tag="qmask_sb")
            s33_sb = cpool.tile([128, T], bf16, tag="s33_sb")
            qT_sb = cpool.tile([128, T], bf16, tag="qT_sb")
            qT2_sb = cpool.tile([128, T], bf16, tag="qT2_sb")
            kgs_sb = [cpool.tile([128, 1024], bf16, name=f"kgs{st}",
                                 tag=f"kgs{st}") for st in range(2)]
            w_colb = cpool.tile([128, T // 128], bf16, tag="w_colb")
            w_col = cpool.tile([128, T // 128], f32, tag="w_col")
            warm_sb = cpool.tile([128, 512], bf16, tag="warm_sb")

            kin = [dpool.tile([IDX, 1024], bf16, name=f"kin{st}",
                              tag=f"kin{st}") for st in range(2)]
            kg = [dpool.tile([2 * IDX, 1024], bf16, name=f"kg{st}",
                             tag=f"kg{st}") for st in range(2)]
            dmy_in = dpool.tile([IDX, 2], bf16, name="dmy_in", tag="dmy_in")
            dmy_out = dpool.tile([2 * IDX, 2], bf16, name="dmy_out",
                                 tag="dmy_out")

            # engine ops at a nonzero partition offset may span at most
            # 32 partitions — memset those regions in 32-row chunks
            nc.vector.memset(warm_sb[:], 0)
            nc.vector.memset(qT_sb[:], 0)
            nc.vector.memset(qT2_sb[:], 0)
            for p0 in range(32, 128, 32):
                nc.vector.memset(s33_sb[p0:p0 + 32, :], 0)
                for st in range(2):
                    nc.vector.memset(kgs_sb[st][p0:p0 + 32, :], 0)

            # dep-free flush collective: completes the one-time all-core
            # CC-stream entry barrier at t~8us instead of attaching it
            # to the first real AllGather (whose doorbell waits on the
            # input pipeline of the slowest core)
            nc.gpsimd.dma_start(out=dmy_in[:], in_=warm_sb[0:IDX, 0:2])
            nc.gpsimd.collective_compute(
                "AllGather",
                mybir.AluOpType.bypass,
                replica_groups=groups,
                ins=[dmy_in.opt()],
                outs=[dmy_out.opt()],
            )

            # PE warm-up bridges the first input slab's load time
            if WARM_N:
                pwarm = pw_pool.tile([128, 512], f32, tag="pwarm")
                for wi in range(WARM_N):
                    nc.tensor.matmul(
                        pwarm[:], warm_sb[:, 0:128], warm_sb[:],
                        start=(wi == 0), stop=(wi == WARM_N - 1),
                    )

            # small persistent loads on the ACT ring (ahead of slab
            # halves so they land first)
            nc.scalar.dma_start(out=wkqw_sb[:], in_=wkqw)
            nc.scalar.dma_start(out=qmask_sb[:], in_=qmask)

            # input slabs: halves alternate across the two HWDGE rings;
            # ONLY bulk bytes ride these rings (small extracts would be
            # FIFO-blocked behind megabytes of queued input)
            slabs = []
            half = DC * 256
            for g in range(NG):
                slab = slab_pool.tile([128, DC * 512], bf16, tag="slab")
                nc.sync.dma_start(
                    out=slab[:, 0:half],
                    in_=xh[g * 128:(g + 1) * 128, 0:half])
                nc.scalar.dma_start(
                    out=slab[:, half:2 * half],
                    in_=xh[g * 128:(g + 1) * 128, half:2 * half])
                slabs.append(slab)

            # --- emission helpers ---
            st_gi = [0]

            def proj_mm(g):
                slab_v = slabs[g][:].rearrange("p (kd t) -> p kd t", kd=DC)
                pj = pj_pool.tile([W33, 512], f32, tag="pj")
                for kd in range(DC):
                    nc.tensor.matmul(
                        pj[:],
                        wkqw_sb[:, kd * W33:(kd + 1) * W33],
                        slab_v[:, kd, :],
                        start=(kd == 0), stop=(kd == DC - 1),
                    )
                c0, c1 = g * 512, (g + 1) * 512
                nc.vector.tensor_copy(s33_sb[0:W33, c0:c1], pj[:])

            def qT_extract(g):
                # engine reads need 32-aligned partition offsets; DMAs
                # don't — pull q rows out of s33 by SWDGE
                c0, c1 = g * 512, (g + 1) * 512
                nc.gpsimd.dma_start(
                    out=qT_sb[0:IDX, c0:c1], in_=s33_sb[IDX:2 * IDX, c0:c1])

            def kin_extract(g):
                c0, c1 = g * 512, (g + 1) * 512
                st, hf = g // 2, g % 2
                nc.gpsimd.dma_start(
                    out=kin[st][:, hf * 512:(hf + 1) * 512],
                    in_=s33_sb[0:IDX, c0:c1])

            def w_extract(g):
                c0 = g * 512
                for gi_ in range(4):
                    t0 = c0 + gi_ * 128
                    nc.gpsimd.dma_start(
                        out=w_colb[:, g * 4 + gi_:g * 4 + gi_ + 1],
                        in_=s33_sb[2 * IDX:W33, t0:t0 + 128],
                    )

            def qT2_extract(g):
                c0, c1 = g * 512, (g + 1) * 512
                nc.gpsimd.dma_start(
                    out=qT2_sb[0:IDX, c0:c1], in_=s33_sb[IDX:2 * IDX, c0:c1])
                nc.gpsimd.dma_start(
                    out=qT2_sb[IDX:2 * IDX, c0:c1],
                    in_=s33_sb[IDX:2 * IDX, c0:c1])

            def w_cast(g):
                nc.vector.tensor_copy(
                    w_col[:, g * 4:(g + 1) * 4],
                    w_colb[:, g * 4:(g + 1) * 4])

            def collective(st):
                nc.gpsimd.collective_compute(
                    "AllGather",
                    mybir.AluOpType.bypass,
                    replica_groups=groups,
                    ins=[kin[st].opt()],
                    outs=[kg[st].opt()],
                )

            def kgs_load(st):
                # SWDGE so no HWDGE ring (and no compute engine's NX)
                # blocks on the collective-done wait
                nc.gpsimd.dma_start(
                    out=kgs_sb[st][0:2 * IDX, :], in_=kg[st][:])

            def qT2_mask(g):
                # sibling-slot select folded into the B-stationary:
                # zero the q copy in the own-member row slot via the
                # per-core 0/1 mask (data-driven, program uniform)
                c0, c1 = g * 512, (g + 1) * 512
                nc.vector.tensor_scalar_mul(
                    out=qT2_sb[0:2 * IDX, c0:c1],
                    in0=qT2_sb[0:2 * IDX, c0:c1],
                    scalar1=qmask_sb[:, 0:1],
                )

            def score_block(i, stat, rhs, rb, ob, ring):
                ps = ps_pool.tile([128, 1024], f32, tag="ps")
                for jj in range(2):
                    j0 = rb + jj * 512
                    nc.tensor.matmul(
                        ps[:, jj * 512:(jj + 1) * 512],
                        stat[:, i * 128:(i + 1) * 128],
                        rhs[:, j0:j0 + 512],
                        start=True, stop=True,
                    )
                osb = out_pool.tile([128, 1024], bf16, tag="osb")
                gi = st_gi[0]
                st_gi[0] += 1
                if gi % FUSE_MOD == 0:
                    nc.vector.tensor_scalar(
                        out=osb[:],
                        in0=ps[:],
                        scalar1=0.0,
                        scalar2=w_col[:, i:i + 1],
                        op0=mybir.AluOpType.max,
                        op1=mybir.AluOpType.mult,
                    )
                else:
                    nc.scalar.activation(
                        osb[:], ps[:], mybir.ActivationFunctionType.Relu)
                    nc.vector.tensor_scalar_mul(
                        out=osb[:], in0=osb[:], scalar1=w_col[:, i:i + 1])
                ring.dma_start(
                    out=o[i * 128:(i + 1) * 128, ob:ob + 1024],
                    in_=osb[:],
                )

            # --- program ---
            # PE:     warm | g0 | g1 | P1a | g2 | P1b | P2a | g3 | P2b
            #         | P3 | P4 | P5
            # GPSIMD: dummy-CC | qT0 kin0 qT2(0) w0 | qT1 kin1 CC0
            #         qT2(1) w1 | qT2e kin2 qT2(2) w2 | qT3 kin3 CC1
            #         qT2(3) w3 | kgs0 kgs1
            # sync:   slabs-a | P1a P1b P2a P2b P4 outs
            # scalar: wkqw qmask slabs-b | P3 P5 outs
            proj_mm(0)
            qT_extract(0); kin_extract(0); qT2_extract(0); w_extract(0)
            w_cast(0)

            proj_mm(1)
            qT_extract(1); kin_extract(1)
            collective(0)
            qT2_extract(1)

            for i in range(0, 4):                       # P1a: A0 i0-3
                score_block(i, qT_sb, s33_sb, 0, 0, nc.sync)
            w_extract(1); w_cast(1)

            proj_mm(2)
            qT_extract(2); kin_extract(2); qT2_extract(2); w_extract(2)

            for i in range(4, 8):                       # P1b: A0 i4-7
                score_block(i, qT_sb, s33_sb, 0, 0, nc.sync)
            w_cast(2)

            for i in range(8, 12):                      # P2a: A0 i8-11
                score_block(i, qT_sb, s33_sb, 0, 0, nc.sync)

            proj_mm(3)
            qT_extract(3); kin_extract(3)
            collective(1)
            qT2_extract(3); w_extract(3)
            w_cast(3)
            for g in range(4):
                qT2_mask(g)
            kgs_load(0)
            kgs_load(1)

            for i in range(12, 16):                     # P2b: A0 i12-15
                score_block(i, qT_sb, s33_sb, 0, 0, nc.sync)
            for i in range(16):                         # P3: A1
                score_block(i, qT_sb, s33_sb, 1024, 1024, nc.scalar)
            for i in range(16):                         # P4: B0
                score_block(i, qT2_sb, kgs_sb[0], 0, 2048, nc.sync)
            for i in range(16):                         # P5: B1
                score_block(i, qT2_sb, kgs_sb[1], 0, 3072, nc.scalar)

    nc.compile()
    _CACHE["nc"] = nc
    return nc


def _make_in_maps(x, Wq, Wk, Ww):
    import ml_dtypes
    bf = ml_dtypes.bfloat16
    w33 = np.concatenate([Wk, Wq, Ww], axis=1).astype(bf)       # [D, 33]
    wkqw = np.ascontiguousarray(
        w33.reshape(DC, 128, W33).transpose(1, 0, 2).reshape(128, DC * W33))
    xbf = x.astype(bf)
    in_maps = []
    for c in range(N_CORES):
        b, h = c // 2, c % 2
        own = xbf[b, h * T:(h + 1) * T, :]                       # [T, D]
        xt = own.T                                               # [D, T]
        xs = np.ascontiguousarray(
            xt.reshape(DC, 128, NG, 512)
            .transpose(2, 1, 0, 3).reshape(NG * 128, DC * 512))
        qm = np.zeros((2 * IDX, 1), dtype=np.float32)
        qm[(1 - h) * IDX:(2 - h) * IDX, 0] = 1.0   # keep sibling's rows
        in_maps.append({"xh": xs, "wkqw": wkqw, "qmask": qm})
    return in_maps


def _assemble(results):
    out = np.empty((B, S, S), dtype=np.float32)
    for c in range(N_CORES):
        b, h = c // 2, c % 2
        oc = np.asarray(results[c]["o"], dtype=np.float32)
        r0 = h * T
        out[b, r0:r0 + T, h * T:(h + 1) * T] = oc[:, 0:T]
        out[b, r0:r0 + T, (1 - h) * T:(2 - h) * T] = oc[:, T:S]
    return out


def kernel(x, Wq, Wk, Ww, _trace_kwargs=None):
    nc = _build_nc()
    in_maps = _make_in_maps(np.asarray(x, dtype=np.float32),
                            np.asarray(Wq, dtype=np.float32),
                            np.asarray(Wk, dtype=np.float32),
                            np.asarray(Ww, dtype=np.float32))
    kw = _trace_kwargs or {}
    res = run_bass_kernel_spmd(nc, in_maps, list(range(N_CORES)), **kw)
    out = _assemble(res.results)
    if _trace_kwargs is not None:
        return out, res
    return out
